# revision 1
# baseline (speedup 1.0000x reference)
"""Trainium2 Bass kernel for nn_CrossProduct (factorization-machine cross term).

out = 0.5 * sum_n [(x @ v)^2 - (x^2) @ (v^2)]   per row, shape (B, 1)

Math restructuring:
  sum_n (x^2 @ v^2)[b, n] = x_b^2 . w   with w = rowsum(v^2)  (1024,)
  => out_b = sum_n (x_b @ (v/sqrt2))^2  +  x_b^2 . (-0.5 w)

Distribution: pure data-parallel over batch across 8 NeuronCores
(2048 rows/core); vparam-derived weights replicated.

Per-core device program (bf16 inputs, fp32 PSUM accumulation):
  - x shipped pre-transposed/chunked from host as XT[p, m, c, b'] =
    x[m*512+b', c*128+p] in bf16 (k on partitions -> natural matmul lhs/rhs).
  - per b-tile m (4 x 512 cols):
      * square xt on DVE+GpSimd -> x2 (bf16)
      * PE: psumA[64,512]   += v_c'.T @ xt_c      (term 1, cols 0-63 of PE)
            psumO[64:65,:]  += wneg_c.T @ x2_c    (term 2, col 64, concurrent)
      * DVE: sq = psumA^2 (fp32)
      * PE: psumO[64:65,:] += ones.T @ sq  (f32r, 1 cyc/row)  == final out row
      * ACT: copy psumO -> out SBUF row
  - single 8KB DMA of the 2048 outputs.
"""

import math
from contextlib import ExitStack

import ml_dtypes
import numpy as np

import concourse.bass as bass
import concourse.bacc as bacc
import concourse.mybir as mybir
import concourse.tile as tile
from concourse.bass_utils import run_bass_kernel_spmd

BF16 = mybir.dt.bfloat16
F32 = mybir.dt.float32
F32R = mybir.dt.float32r

N_CORES = 8
B, XD, KD = 16384, 1024, 64
BS = B // N_CORES  # 2048 batch rows per core
C = XD // 128      # 8 contraction chunks of 128
MT = 4             # b-tiles per core
BT = BS // MT      # 512 batch cols per tile
DVE_CHUNKS = 5     # chunks squared on DVE; rest on GpSimd


def _body(ctx, tc, OUT, XT, VW):
    nc = tc.nc
    const = ctx.enter_context(tc.tile_pool(name="const", bufs=1))
    xpool = ctx.enter_context(tc.tile_pool(name="xp", bufs=4))
    x2apool = ctx.enter_context(tc.tile_pool(name="x2a", bufs=4))
    x2bpool = ctx.enter_context(tc.tile_pool(name="x2b", bufs=4))
    sqpool = ctx.enter_context(tc.tile_pool(name="sqp", bufs=4))
    opool = ctx.enter_context(tc.tile_pool(name="op", bufs=1))
    psA = ctx.enter_context(tc.tile_pool(name="psA", bufs=4, space="PSUM"))
    psO = ctx.enter_context(tc.tile_pool(name="psO", bufs=4, space="PSUM"))

    # vw columns per chunk c: [0:64]=v/sqrt2, 64=-0.5*w, 65=1.0 (reduce
    # weights), 66=0.0 (activation bias source) -- one DMA, so every PE
    # weight load is covered by a single already-observed semaphore.
    vw = const.tile([128, C, 67], BF16)
    nc.scalar.dma_start(vw[:], VW)
    outs = opool.tile([65, BS], F32)
    # one-time ACT touch of the vw DMA so later Square ops (which read the
    # bias column) carry only their PE wait (1-wait ISA limit per inst).
    actwarm = const.tile([128, 1], BF16)
    nc.scalar.copy(actwarm[:], vw[:, 0, 66:67])

    for m in range(MT):
        xt = xpool.tile([128, C, BT], BF16)
        # alternate the two HWDGE rings (SP / ACT) to double load bandwidth
        (nc.sync if m % 2 == 0 else nc.scalar).dma_start(xt[:], XT[:, m])
        x2a = x2apool.tile([128, DVE_CHUNKS, BT], BF16)
        nc.vector.tensor_mul(
            x2a[:], xt[:, 0:DVE_CHUNKS], xt[:, 0:DVE_CHUNKS]
        )
        x2b = x2bpool.tile([128, C - DVE_CHUNKS, BT], BF16)
        nc.gpsimd.tensor_mul(
            x2b[:], xt[:, DVE_CHUNKS:C], xt[:, DVE_CHUNKS:C]
        )

        pa = psA.tile([64, BT], F32)
        po = psO.tile([65, BT], F32)
        for c in range(C):
            nc.tensor.matmul(
                pa[:],
                vw[:, c, 0:64],
                xt[:, c],
                start=(c == 0),
                stop=(c == C - 1),
                tile_position=(0, 0),
            )
            nc.tensor.matmul(
                po[64:65, :],
                vw[:, c, 64:65],
                x2a[:, c] if c < DVE_CHUNKS else x2b[:, c - DVE_CHUNKS],
                start=(c == 0),
                stop=False,
                tile_position=(0, 64),
            )
        sq = sqpool.tile([64, BT], BF16)
        nc.scalar.activation(
            sq[:],
            pa[:],
            mybir.ActivationFunctionType.Square,
            bias=vw[0:64, 0, 66:67],
        )
        nc.tensor.matmul(
            po[64:65, :],
            vw[0:64, 0, 65:66],
            sq[:],
            start=False,
            stop=True,
            tile_position=(0, 64),
        )
        nc.scalar.copy(outs[64:65, m * BT : (m + 1) * BT], po[64:65, :])

    nc.sync.dma_start(OUT, outs[64:65, :])


_NC_CACHE = None


def build_nc():
    global _NC_CACHE
    if _NC_CACHE is not None:
        return _NC_CACHE
    nc = bacc.Bacc("TRN2", target_bir_lowering=False, debug=False)
    XT = nc.dram_tensor("XT", [128, MT, C, BT], BF16, kind="ExternalInput").ap()
    VW = nc.dram_tensor("VW", [128, C, 67], BF16, kind="ExternalInput").ap()
    OUT = nc.dram_tensor("OUT", [1, BS], F32, kind="ExternalOutput").ap()
    with tile.TileContext(nc) as tc:
        with ExitStack() as ctx:
            _body(ctx, tc, OUT, XT, VW)
    nc.compile()
    _NC_CACHE = nc
    return nc


def make_in_maps(x, vparam):
    bf = ml_dtypes.bfloat16
    x = np.ascontiguousarray(x, dtype=np.float32)
    v = np.ascontiguousarray(vparam, dtype=np.float32)

    vs = (v / math.sqrt(2.0)).astype(bf)             # (1024, 64)
    w = (v.astype(np.float64) ** 2).sum(axis=1)
    wneg = (-0.5 * w).astype(np.float32).astype(bf)  # (1024,)

    VWh = np.empty((128, C, 67), dtype=bf)
    VWh[:, :, 0:64] = vs.reshape(C, 128, KD).transpose(1, 0, 2)
    VWh[:, :, 64] = wneg.reshape(C, 128).T
    VWh[:, :, 65] = bf(1.0)
    VWh[:, :, 66] = bf(0.0)

    in_maps = []
    for i in range(N_CORES):
        xs = x[i * BS : (i + 1) * BS]                # (2048, 1024)
        xt = np.ascontiguousarray(xs.T)              # (1024, 2048) [k, b]
        # A[p, m, c, b'] = xt[c*128+p, m*512+b']
        A = xt.reshape(C, 128, MT, BT).transpose(1, 2, 0, 3)
        XTh = np.ascontiguousarray(A).astype(bf)
        in_maps.append({"XT": XTh, "VW": VWh})
    return in_maps


LAST_RESULTS = None  # stashed BassKernelResults (for test harness profiling)
TRACE = False


def kernel(x, vparam):
    global LAST_RESULTS
    nc = build_nc()
    in_maps = make_in_maps(x, vparam)
    res = run_bass_kernel_spmd(nc, in_maps, list(range(N_CORES)), trace=TRACE)
    LAST_RESULTS = res
    out = np.concatenate(
        [res.results[i]["OUT"].reshape(BS, 1) for i in range(N_CORES)], axis=0
    )
    return out.astype(np.float32)



# revision 9
# speedup vs baseline: 1.1306x; 1.1306x over previous
"""Trainium2 Bass kernel for nn_CrossProduct (factorization-machine cross term).

out_b = 0.5 * [ sum_k (x_b @ v_k)^2  -  sum_i w_i x_bi^2 ],  w_i = sum_k v_ik^2

Host-side rescaling removes all per-feature weights from the device:
  x'  = x * sqrt(w/2)          (shipped fp16, feature-on-partition, chunk-major)
  v'' = v / sqrt(w)            (replicated fp16)
  => psA[k,b] = x'_b @ v''_k = (x v_k)/sqrt(2);  sq = psA^2 = (xv)^2/2
     term2_b  = sum_i x'_bi^2 = 0.5 sum_i w_i x_bi^2  (constant -1 PE weights!)
  out_b = (ones64 . sq) - term2_b   accumulated in one PSUM row.

Device program per core (2048 rows, 8 contraction chunks of 128):
  - DMA chunk-major halves on two rings (sync / gpsimd queues).
  - PE: per chunk ONE fused 2048-row matmul pa(c) into psA[64,2048] (banks 0-3)
        interleaved with one ones-weight matmul po(c) on x'^2 into psO[64:65]
        (banks 4-7, PE column 64 co-resident with pa's columns 0:63).
        Tensor queue: pa0 pa1 po0 pa2 po1 ... pa7 po6 po7 sqMMa sqMMb.
  - squares x'^2: DVE chunks 0-4, ACT 5-6, GpSimd 7 (off critical path).
  - psA squares: ACT first half / DVE second half -> sq fp16, then two
    ones64-weight matmuls add term1 into psO row 64.
  - copy psO row to SBUF (ACT/DVE halves), single 4KB DMA out (fp16).
"""

import math
from contextlib import ExitStack

import ml_dtypes
import numpy as np

import concourse.bass as bass
import concourse.bacc as bacc
import concourse.mybir as mybir
import concourse.tile as tile
from concourse.bass_utils import run_bass_kernel_spmd

F16 = mybir.dt.float16
F32 = mybir.dt.float32

N_CORES = 8
B, XD, KD = 16384, 1024, 64
BS = B // N_CORES   # 2048 batch rows per core
C = XD // 128       # 8 contraction chunks of 128
H = BS // 2         # 1024 half-batch columns per DMA transfer

DVE_SQ = (0, 1, 2, 3, 4, 5)  # chunks squared on DVE
ACT_SQ = (6,)                # chunks squared on ACT
GPS_SQ = (7,)                # chunks squared on GpSimd


def _body(ctx, tc, OUT, X, VW):
    nc = tc.nc
    const = ctx.enter_context(tc.tile_pool(name="const", bufs=1))
    xpool = ctx.enter_context(tc.tile_pool(name="xp", bufs=1))
    x2pool = ctx.enter_context(tc.tile_pool(name="x2p", bufs=1))
    sqpool = ctx.enter_context(tc.tile_pool(name="sqp", bufs=1))
    opool = ctx.enter_context(tc.tile_pool(name="op", bufs=1))
    psa = ctx.enter_context(tc.tile_pool(name="psA", bufs=1, space="PSUM"))
    pso = ctx.enter_context(tc.tile_pool(name="psO", bufs=1, space="PSUM"))

    # vw cols: [c*64:(c+1)*64] = v''_c; col 512 = -1 (term2 weights);
    # col 513 = +1 (term1 reduce weights).
    vw = const.tile([128, C * KD + 2], F16)
    nc.scalar.dma_start(vw[:], VW)

    xt = xpool.tile([128, C, BS], F16)
    for c in range(C):
        nc.sync.dma_start(xt[:, c, 0:H], X[c, 0])
        nc.gpsimd.dma_start(xt[:, c, H:BS], X[c, 1])

    x2 = x2pool.tile([128, C, BS], F16)
    for c in DVE_SQ:
        nc.vector.tensor_mul(x2[:, c], xt[:, c], xt[:, c])
    for c in ACT_SQ:
        nc.scalar.activation(
            x2[:, c], xt[:, c], mybir.ActivationFunctionType.Square
        )
    for c in GPS_SQ:
        nc.gpsimd.tensor_mul(x2[:, c], xt[:, c], xt[:, c])

    pa = psa.tile([64, BS], F32)
    po = pso.tile([65, BS], F32)

    # matmul output must stay within one PSUM bank -> 512-column splits
    def pa_mm(c):
        for q in range(4):
            nc.tensor.matmul(
                pa[:, q * 512 : (q + 1) * 512],
                vw[:, c * KD : (c + 1) * KD],
                xt[:, c, q * 512 : (q + 1) * 512],
                start=(c == 0),
                stop=(c == C - 1),
                tile_position=(0, 0),
            )

    def po_mm(c):
        for q in range(4):
            nc.tensor.matmul(
                po[64:65, q * 512 : (q + 1) * 512],
                vw[:, C * KD : C * KD + 1],
                x2[:, c, q * 512 : (q + 1) * 512],
                start=(c == 0),
                stop=False,
                tile_position=(0, 64),
            )

    # pa leads po by 2 chunks so po never stalls the tensor queue.
    pa_mm(0)
    for c in range(1, C):
        pa_mm(c)
        po_mm(c - 1)
    po_mm(C - 1)

    # term1: square psA (halves on ACT / DVE), reduce over k via ones64.
    sq = sqpool.tile([64, BS], F16)
    for q in range(4):
        nc.scalar.activation(
            sq[:, q * 512 : (q + 1) * 512],
            pa[:, q * 512 : (q + 1) * 512],
            mybir.ActivationFunctionType.Square,
        )
    ones64 = vw[0:64, C * KD + 1 : C * KD + 2]
    for q in range(4):
        nc.tensor.matmul(
            po[64:65, q * 512 : (q + 1) * 512],
            ones64,
            sq[:, q * 512 : (q + 1) * 512],
            start=False,
            stop=True,
            tile_position=(0, 64),
        )

    outs = opool.tile([65, BS], F16)
    for q in range(4):
        dst = outs[64:65, q * 512 : (q + 1) * 512]
        src = po[64:65, q * 512 : (q + 1) * 512]
        if q % 2 == 0:
            nc.scalar.copy(dst, src)
        else:
            nc.vector.tensor_scalar_mul(dst, src, 1.0)
    nc.sync.dma_start(OUT, outs[64:65, :])


_NC_CACHE = None


def build_nc():
    global _NC_CACHE
    if _NC_CACHE is not None:
        return _NC_CACHE
    nc = bacc.Bacc("TRN2", target_bir_lowering=False, debug=False)
    X = nc.dram_tensor("X", [C, 2, 128, H], F16, kind="ExternalInput").ap()
    VW = nc.dram_tensor("VW", [128, C * KD + 2], F16, kind="ExternalInput").ap()
    OUT = nc.dram_tensor("OUT", [1, BS], F16, kind="ExternalOutput").ap()
    with tile.TileContext(nc) as tc:
        with ExitStack() as ctx:
            _body(ctx, tc, OUT, X, VW)
    nc.compile()
    _NC_CACHE = nc
    return nc


def make_in_maps(x, vparam):
    f16 = ml_dtypes.float16 if hasattr(ml_dtypes, "float16") else np.float16
    x = np.ascontiguousarray(x, dtype=np.float32)
    v = np.ascontiguousarray(vparam, dtype=np.float32)

    w = (v.astype(np.float64) ** 2).sum(axis=1)          # (1024,)
    w = np.maximum(w, 1e-12)
    s = np.sqrt(w / 2.0)                                 # x scale
    vn = (v / np.sqrt(w)[:, None]).astype(np.float32)    # (1024, 64)

    VWh = np.empty((128, C * KD + 2), dtype=np.float16)
    # VW[p, c*64+k] = vn[c*128+p, k]
    VWh[:, 0 : C * KD] = (
        vn.reshape(C, 128, KD).transpose(1, 0, 2).reshape(128, C * KD)
    )
    VWh[:, C * KD] = -1.0
    VWh[:, C * KD + 1] = 1.0

    xs_all = (x * s[None, :]).astype(np.float16)         # (B, 1024)

    in_maps = []
    for i in range(N_CORES):
        xs = xs_all[i * BS : (i + 1) * BS]               # (2048, 1024)
        # X[c, h, p, j] = xs.T[c*128+p, h*1024+j]
        A = xs.T.reshape(C, 128, 2, H).transpose(0, 2, 1, 3)
        in_maps.append({"X": np.ascontiguousarray(A), "VW": VWh})
    return in_maps


LAST_RESULTS = None  # stashed BassKernelResults (for test harness profiling)
TRACE = False


def kernel(x, vparam):
    global LAST_RESULTS
    nc = build_nc()
    in_maps = make_in_maps(x, vparam)
    res = run_bass_kernel_spmd(nc, in_maps, list(range(N_CORES)), trace=TRACE)
    LAST_RESULTS = res
    out = np.concatenate(
        [
            res.results[i]["OUT"].astype(np.float32).reshape(BS, 1)
            for i in range(N_CORES)
        ],
        axis=0,
    )
    return out.astype(np.float32)
